# revision 4
# baseline (speedup 1.0000x reference)
"""Trainium2 Bass kernel for AudioPreprocessingLayer.

Computes: floor(log2(mel_fb @ (rfft(x*hamming, norm=forward).real ** 2)))
for x of shape (4096, 32, 512), sharded batch-wise across 8 NeuronCores.

Key ideas:
  - rfft(.).real is a matmul with the cosine matrix C[n,k] = cos(2*pi*k*n/512)/512.
    The hamming window folds into it host-side: W = diag(hw) @ C, stored bf16.
  - Mel filterbank column 0 (DC bin) is structurally zero, so only bins 1..256
    are computed -> 256 = 2x128 clean chunks (checked at runtime, with a
    257-bin fallback).
  - x is cast once to bf16 by the load DMA; the on-chip transpose runs on the
    DMA XBAR (InstDmaTransposeAnt): ONE instruction transposes a whole macro
    [128p, JT, 512n] -> [128 n', (j,q), 128 p], so the PE does no transpose
    matmuls and no PSUM->SBUF copies are needed at all.
  - floor(log2(m)) for positive fp32 m is exactly
    max(bitcast_int32(m) >> 23, 75) - 127   (the max() also maps the
    mels==0 -> eps=2^-52 case to -52 exactly).
  - Rows are mapped to partitions in blocks of JT per macro-group
    (row = m0 + JT*p + j), so every load DMA descriptor covers JT consecutive
    DRAM rows (up to 32 KB in), and the xbar output ordering lines up with the
    store pattern with no extra shuffling.
"""

import os
import sys

for _p in ("/opt/trn_rl_repo",):
    if _p not in sys.path and os.path.isdir(_p):
        sys.path.append(_p)

import numpy as np
import ml_dtypes

import concourse.bass as bass
from concourse import bacc, mybir
from concourse.tile import TileContext
from concourse.bass_utils import run_bass_kernel_spmd

N_CORES = 8
B, T, FRAME = 4096, 32, 512
R_PER_CORE = (B // N_CORES) * T  # 16384 rows of length 512 per core
N_MELS = 20

f32 = mybir.dt.float32
bf16 = mybir.dt.bfloat16
i32 = mybir.dt.int32


def _ceil_div(a, b):
    return (a + b - 1) // b


def build_graph(R=R_PER_CORE, NF=256, group_r=512):
    """Build the SPMD Bass graph for one core's shard.

    x:   [R, 512]  f32   rows to transform
    w:   [4, 128, NF] bf16  cosine*window matrix, chunked along n
    fbt: [NFC, 128, N_MELS] bf16  mel filterbank transposed+chunked along freq
    out: [R, N_MELS] f32
    """
    assert R % group_r == 0 and group_r % 128 == 0
    RT = group_r // 128          # row subtiles per group (4)
    NQ = FRAME // 128            # 4 n-chunks
    NFC = _ceil_div(NF, 128)     # freq chunks
    f_sizes = [min(128, NF - 128 * c) for c in range(NFC)]

    nc = bacc.Bacc(None, target_bir_lowering=False)
    x_d = nc.declare_dram_parameter("x", [R, FRAME], f32, isOutput=False)
    w_d = nc.declare_dram_parameter("w", [NQ, 128, NF], bf16, isOutput=False)
    fbt_d = nc.declare_dram_parameter("fbt", [NFC, 128, N_MELS], bf16, isOutput=False)
    out_d = nc.declare_dram_parameter("out", [R, N_MELS], f32, isOutput=True)

    with TileContext(nc) as tc:
        with (
            tc.tile_pool(name="consts", bufs=1) as consts,
            tc.tile_pool(name="xb", bufs=3) as xb_pool,
            tc.tile_pool(name="xt", bufs=2) as xt_pool,
            tc.tile_pool(name="mag", bufs=3) as mag_pool,
            tc.tile_pool(name="fin", bufs=3) as fin_pool,
            tc.tile_pool(name="ps_y", bufs=2, space="PSUM") as ps_y_pool,
            tc.tile_pool(name="ps_m", bufs=2, space="PSUM") as ps_m_pool,
        ):
            # ---- constants (bf16 straight from DRAM, no casts) ----
            w_sb = consts.tile([128, NQ, NF], bf16)
            nc.sync.dma_start(out=w_sb, in_=w_d.rearrange("q p f -> p q f"))
            fbt_sb = consts.tile([128, NFC, N_MELS], bf16)
            nc.sync.dma_start(out=fbt_sb, in_=fbt_d.rearrange("c p m -> p c m"))

            # macro-group sizes (in groups of group_r rows): small at the head
            # so the pipeline fills quickly, small at the tail to shrink the
            # drain.
            n_groups = R // group_r
            gpm = 2048 // group_r
            head = [1, 1, 2]
            tail = [max(1, gpm // 2)] * 4
            mid = n_groups - sum(head) - sum(tail)
            if mid >= gpm:
                body = [gpm] * (mid // gpm)
                if mid % gpm:
                    body.append(mid % gpm)
                gpm_list = head + body + tail
            else:
                gpm_list = [1] * n_groups
            assert sum(gpm_list) == n_groups, (gpm_list, n_groups)

            groups = []   # (macro, gg) per group
            macros = []   # per macro: dict(m0, GPM, JT)
            m0 = 0
            for mg, GPM in enumerate(gpm_list):
                macros.append({"m0": m0, "GPM": GPM, "JT": GPM * RT})
                for gg in range(GPM):
                    groups.append((mg, gg))
                m0 += GPM * group_r

            st = {}

            def stage_T(g):
                mg, gg = groups[g]
                mac = macros[mg]
                if gg == 0:
                    JT = mac["JT"]
                    # casting DMA (f32 dram -> bf16 sbuf); row m0 + JT*p + j
                    # -> partition p, slot j (big contiguous descriptors)
                    xb_sb = xb_pool.tile([128, JT, FRAME], bf16, name="xb_sb")
                    nc.gpsimd.dma_start(
                        out=xb_sb,
                        in_=x_d[
                            mac["m0"] : mac["m0"] + JT * 128, :
                        ].rearrange("(p j) n -> p j n", j=JT),
                    )
                    # ONE xbar transpose for the whole macro:
                    # [128 p, JT*512] -> out[n', (j, q), p] with
                    # n = q*128 + n', flat col j*512 + n.
                    xt_sb = xt_pool.tile([128, JT, NQ, 128], bf16, name="xt_sb")
                    nc.sync.dma_start(out=xt_sb, in_=xb_sb, transpose=True)
                    mac["xt"] = xt_sb
                    mac["e_sb"] = fin_pool.tile(
                        [128, JT * N_MELS], i32, tag="e_sb", name="e_sb"
                    )

            def stage_M1(g):
                # matmul 1: yT[f, r] += W[n, f].T @ xT[n, r]; then square
                mg, gg = groups[g]
                mac = macros[mg]
                xt_sb = mac["xt"]
                y_ps = ps_y_pool.tile([128, NFC, group_r], f32, name="y_ps")
                for c in range(NFC):
                    fs = f_sizes[c]
                    for q in range(NQ):
                        nc.tensor.matmul(
                            y_ps[:fs, c, :],
                            w_sb[:, q, 128 * c : 128 * c + fs],
                            xt_sb[:, gg * RT : (gg + 1) * RT, q, :],
                            start=(q == 0),
                            stop=(q == NQ - 1),
                        )
                # square: magT = yT*yT (fused, psum -> sbuf bf16)
                mag_sb = mag_pool.tile([128, NFC, group_r], bf16, name="mag_sb")
                nc.scalar.activation(
                    mag_sb, y_ps, mybir.ActivationFunctionType.Square
                )
                st[("mag", g)] = mag_sb

            def stage_M2(g):
                mg, gg = groups[g]
                mac = macros[mg]
                mag_sb = st.pop(("mag", g))
                # matmul 2: mels[r, m] += magT[f, r].T @ fbt[f, m]
                mels_ps = ps_m_pool.tile([128, RT * N_MELS], f32, name="mels_ps")
                for j in range(RT):
                    for c in range(NFC):
                        fs = f_sizes[c]
                        nc.tensor.matmul(
                            mels_ps[:, j * N_MELS : (j + 1) * N_MELS],
                            mag_sb[:fs, c, j * 128 : (j + 1) * 128],
                            fbt_sb[:fs, c, :],
                            start=(c == 0),
                            stop=(c == NFC - 1),
                        )
                # exponent bits out of PSUM (rest of finalize is batched)
                nc.vector.tensor_scalar(
                    mac["e_sb"][:, gg * RT * N_MELS : (gg + 1) * RT * N_MELS],
                    mels_ps.bitcast(i32),
                    23,
                    None,
                    mybir.AluOpType.logical_shift_right,
                )
                if gg == mac["GPM"] - 1:
                    # finalize: floor(log2(m)) = max(bits >> 23, 75) - 127
                    JT = mac["JT"]
                    e_sb = mac["e_sb"]
                    ef_sb = fin_pool.tile(
                        [128, JT * N_MELS], f32, tag="ef_sb", name="ef_sb"
                    )
                    nc.vector.tensor_copy(ef_sb, e_sb)
                    o_sb = fin_pool.tile(
                        [128, JT * N_MELS], f32, tag="o_sb", name="o_sb"
                    )
                    nc.vector.tensor_scalar(
                        o_sb,
                        ef_sb,
                        75.0,
                        127.0,
                        mybir.AluOpType.max,
                        mybir.AluOpType.subtract,
                    )
                    # store: one DMA per macro, JT rows per partition
                    nc.scalar.dma_start(
                        out=out_d[
                            mac["m0"] : mac["m0"] + JT * 128, :
                        ].rearrange("(p j) m -> p (j m)", j=JT),
                        in_=o_sb,
                    )

            for g in range(len(groups)):
                stage_T(g)
                stage_M1(g)
                stage_M2(g)
    nc.compile()
    return nc


def _prep_weights(filter_banks, hw):
    """Host-side: cosine*window matrix (bf16) and transposed filterbank."""
    fb = np.asarray(filter_banks, dtype=np.float32)
    n_mels, n_bins = fb.shape  # (20, 257)
    assert n_mels == N_MELS and n_bins == FRAME // 2 + 1

    if np.all(fb[:, 0] == 0.0):
        k0 = 1  # DC bin unused by the filterbank (structurally true)
    else:
        k0 = 0
    NF = n_bins - k0

    n = np.arange(FRAME, dtype=np.float64)
    k = np.arange(k0, n_bins, dtype=np.float64)
    C = np.cos(2.0 * np.pi * np.outer(n, k) / FRAME) / FRAME
    W = (np.asarray(hw, dtype=np.float64)[:, None] * C).astype(ml_dtypes.bfloat16)
    NQ = FRAME // 128
    w_chunks = np.ascontiguousarray(W.reshape(NQ, 128, NF))

    NFC = _ceil_div(NF, 128)
    fbt = np.zeros((NFC, 128, N_MELS), dtype=ml_dtypes.bfloat16)
    fbT = fb[:, k0:].T.astype(ml_dtypes.bfloat16)  # [NF, 20]
    for c in range(NFC):
        fs = min(128, NF - 128 * c)
        fbt[c, :fs, :] = fbT[128 * c : 128 * c + fs, :]
    return w_chunks, fbt, NF


_CACHE = {}


def _get_graph(R, NF, group_r):
    key = (R, NF, group_r)
    if key not in _CACHE:
        _CACHE[key] = build_graph(R, NF, group_r)
    return _CACHE[key]


def kernel(inputs, filter_banks, hw, _trace=False, _group_r=512):
    x = np.ascontiguousarray(np.asarray(inputs, dtype=np.float32))
    assert x.shape == (B, T, FRAME), x.shape
    w_chunks, fbt, NF = _prep_weights(filter_banks, hw)

    shards = x.reshape(N_CORES, B // N_CORES * T, FRAME)
    nc = _get_graph(R_PER_CORE, NF, _group_r)
    in_maps = [
        {"x": shards[i], "w": w_chunks, "fbt": fbt} for i in range(N_CORES)
    ]
    res = run_bass_kernel_spmd(
        nc, in_maps, core_ids=list(range(N_CORES)), trace=_trace
    )
    out = np.stack([res.results[i]["out"] for i in range(N_CORES)], axis=0)
    out = out.reshape(B, T, N_MELS, 1).astype(np.float32)
    if _trace:
        kernel._last_result = res
    return out


# revision 9
# speedup vs baseline: 1.1466x; 1.1466x over previous
"""Trainium2 Bass kernel for AudioPreprocessingLayer.

Computes: floor(log2(mel_fb @ (rfft(x*hamming, norm=forward).real ** 2)))
for x of shape (4096, 32, 512), sharded batch-wise across 8 NeuronCores.

Key ideas:
  - rfft(.).real is a matmul with the cosine matrix C[n,k] = cos(2*pi*k*n/512)/512.
    The hamming window folds into it host-side: W = diag(hw) @ C, stored bf16.
  - Mel filterbank column 0 (DC bin) is structurally zero, so only bins 1..256
    are computed -> 256 = 2x128 clean chunks (checked at runtime, with a
    257-bin fallback).
  - x is cast once to bf16 by the load DMA. The row transpose (rows -> n on
    partitions) is split between two resources that run concurrently:
      * DMA XBAR (InstDmaTransposeAnt) on the sync queue: one instruction
        transposes a whole macro [128p, JT, 512n] -> [128 n', (j,q), 128 p].
      * PE identity matmuls (bf16) + PSUM->SBUF bf16 copies round-robined
        across the vector/scalar/gpsimd engines.
  - floor(log2(m)) for positive fp32 m is exactly
    max(bitcast_int32(m) >> 23, 75) - 127   (the max() also maps the
    mels==0 -> eps=2^-52 case to -52 exactly).
  - Rows are mapped to partitions in blocks of JT per macro-group
    (row = m0 + JT*p + j), so every load DMA descriptor covers JT consecutive
    DRAM rows (up to 32 KB), and both transpose paths produce the same
    r_local = j*128 + p ordering, matching one store DMA per macro.
"""

import os
import sys

for _p in ("/opt/trn_rl_repo",):
    if _p not in sys.path and os.path.isdir(_p):
        sys.path.append(_p)

import numpy as np
import ml_dtypes

import concourse.bass as bass
from concourse import bacc, mybir
from concourse.tile import TileContext
from concourse.bass_utils import run_bass_kernel_spmd
from concourse.masks import make_identity

N_CORES = 8
B, T, FRAME = 4096, 32, 512
R_PER_CORE = (B // N_CORES) * T  # 16384 rows of length 512 per core
N_MELS = 20

f32 = mybir.dt.float32
bf16 = mybir.dt.bfloat16
i32 = mybir.dt.int32


def _ceil_div(a, b):
    return (a + b - 1) // b


def build_graph(R=R_PER_CORE, NF=256, group_r=512, xbar_mod=2):
    """Build the SPMD Bass graph for one core's shard.

    x:   [R, 512]  f32   rows to transform
    w:   [4, 128, NF] bf16  cosine*window matrix, chunked along n
    fbt: [NFC, 128, N_MELS] bf16  mel filterbank transposed+chunked along freq
    out: [R, N_MELS] f32

    Macro mg uses the xbar transpose iff xbar_mod and mg % xbar_mod == 0,
    else the PE transpose path.
    """
    assert R % group_r == 0 and group_r % 128 == 0
    RT = group_r // 128          # row subtiles per group (4)
    NQ = FRAME // 128            # 4 n-chunks
    NFC = _ceil_div(NF, 128)     # freq chunks
    f_sizes = [min(128, NF - 128 * c) for c in range(NFC)]

    nc = bacc.Bacc(None, target_bir_lowering=False)
    x_d = nc.declare_dram_parameter("x", [R, FRAME], f32, isOutput=False)
    w_d = nc.declare_dram_parameter("w", [NQ, 128, NF], bf16, isOutput=False)
    fbt_d = nc.declare_dram_parameter("fbt", [NFC, 128, N_MELS], bf16, isOutput=False)
    out_d = nc.declare_dram_parameter("out", [R, N_MELS], f32, isOutput=True)

    with TileContext(nc) as tc:
        with (
            tc.tile_pool(name="consts", bufs=1) as consts,
            tc.tile_pool(name="xb", bufs=3) as xb_pool,
            tc.tile_pool(name="xt", bufs=2) as xt_pool,
            tc.tile_pool(name="xq", bufs=2) as xq_pool,
            tc.tile_pool(name="mag", bufs=3) as mag_pool,
            tc.tile_pool(name="fin", bufs=3) as fin_pool,
            tc.tile_pool(name="ps_xt", bufs=3, space="PSUM") as ps_xt_pool,
            tc.tile_pool(name="ps_y", bufs=2, space="PSUM") as ps_y_pool,
            tc.tile_pool(name="ps_m", bufs=1, space="PSUM") as ps_m_pool,
        ):
            # ---- constants (bf16 straight from DRAM, no casts) ----
            ident = consts.tile([128, 128], bf16)
            make_identity(nc, ident)
            w_sb = consts.tile([128, NQ, NF], bf16)
            nc.sync.dma_start(out=w_sb, in_=w_d.rearrange("q p f -> p q f"))
            fbt_sb = consts.tile([128, NFC, N_MELS], bf16)
            nc.sync.dma_start(out=fbt_sb, in_=fbt_d.rearrange("c p m -> p c m"))

            # macro-group sizes (in groups of group_r rows): small at the head
            # so the pipeline fills quickly, small at the tail to shrink the
            # drain.
            n_groups = R // group_r
            gpm = 2048 // group_r
            head = [1, 1, 2]
            tail = [max(1, gpm // 2)] * 4
            mid = n_groups - sum(head) - sum(tail)
            if mid >= gpm:
                body = [gpm] * (mid // gpm)
                if mid % gpm:
                    body.append(mid % gpm)
                gpm_list = head + body + tail
            else:
                gpm_list = [1] * n_groups
            assert sum(gpm_list) == n_groups, (gpm_list, n_groups)

            groups = []   # (macro, gg) per group
            macros = []   # per macro: dict(m0, GPM, JT)
            m0 = 0
            for mg, GPM in enumerate(gpm_list):
                use_xbar = bool(xbar_mod) and (mg % xbar_mod == 0)
                macros.append(
                    {"m0": m0, "GPM": GPM, "JT": GPM * RT, "xbar": use_xbar}
                )
                for gg in range(GPM):
                    groups.append((mg, gg))
                m0 += GPM * group_r

            st = {}
            cp_engines = None  # set lazily (round-robin for PSUM->SBUF copies)

            def stage_T(g):
                mg, gg = groups[g]
                mac = macros[mg]
                if gg == 0:
                    JT = mac["JT"]
                    # casting DMA (f32 dram -> bf16 sbuf); row m0 + JT*p + j
                    # -> partition p, slot j (big contiguous descriptors)
                    xb_sb = xb_pool.tile([128, JT, FRAME], bf16, name="xb_sb")
                    nc.gpsimd.dma_start(
                        out=xb_sb,
                        in_=x_d[
                            mac["m0"] : mac["m0"] + JT * 128, :
                        ].rearrange("(p j) n -> p j n", j=JT),
                    )
                    mac["xb"] = xb_sb
                    if mac["xbar"]:
                        # ONE xbar transpose for the whole macro:
                        # xt[n', (j, q), p] with n = q*128 + n'
                        xt_sb = xt_pool.tile(
                            [128, JT, NQ, 128], bf16, name="xt_sb"
                        )
                        nc.sync.dma_start(out=xt_sb, in_=xb_sb, transpose=True)
                        mac["xt"] = xt_sb
                    mac["e_sb"] = fin_pool.tile(
                        [128, JT * N_MELS], i32, tag="e_sb", name="e_sb"
                    )
                if not mac["xbar"]:
                    # PE transpose of this group's RT row-blocks; PSUM->SBUF
                    # bf16 copies round-robin across vector/scalar/gpsimd
                    nonlocal cp_engines
                    if cp_engines is None:
                        # gpsimd cannot read PSUM; vector gets 2/3 since
                        # scalar also runs the Square activations
                        cp_engines = [nc.vector, nc.scalar, nc.vector]
                    xb_sb = mac["xb"]
                    xq_sb = []
                    for q in range(NQ):
                        t = ps_xt_pool.tile(
                            [128, group_r], f32, name=f"xt{q}", tag="xt"
                        )
                        for j in range(RT):
                            nc.tensor.matmul(
                                t[:, j * 128 : (j + 1) * 128],
                                xb_sb[:, gg * RT + j, q * 128 : (q + 1) * 128],
                                ident,
                                start=True,
                                stop=True,
                            )
                        dst = xq_pool.tile(
                            [128, group_r], bf16, name=f"xq{q}", tag=f"xq{q}"
                        )
                        xq_sb.append(dst)
                        eng = cp_engines[(g * NQ + q) % 3]
                        if eng is nc.scalar:
                            eng.copy(dst, t)
                        else:
                            eng.tensor_copy(dst, t)
                    st[("xq", g)] = xq_sb

            def stage_M1(g):
                # matmul 1: yT[f, r] += W[n, f].T @ xT[n, r]
                mg, gg = groups[g]
                mac = macros[mg]
                y_ps = ps_y_pool.tile([128, NFC, group_r], f32, name="y_ps")
                if mac["xbar"]:
                    xt_sb = mac["xt"]
                    movings = [
                        xt_sb[:, gg * RT : (gg + 1) * RT, q, :] for q in range(NQ)
                    ]
                else:
                    movings = st.pop(("xq", g))
                for c in range(NFC):
                    fs = f_sizes[c]
                    for q in range(NQ):
                        nc.tensor.matmul(
                            y_ps[:fs, c, :],
                            w_sb[:, q, 128 * c : 128 * c + fs],
                            movings[q],
                            start=(q == 0),
                            stop=(q == NQ - 1),
                        )
                # square: magT = yT*yT (fused, psum -> sbuf bf16)
                mag_sb = mag_pool.tile([128, NFC, group_r], bf16, name="mag_sb")
                nc.scalar.activation(
                    mag_sb, y_ps, mybir.ActivationFunctionType.Square
                )
                st[("mag", g)] = mag_sb

            def stage_M2(g):
                mg, gg = groups[g]
                mac = macros[mg]
                mag_sb = st.pop(("mag", g))
                # matmul 2: mels[r, m] += magT[f, r].T @ fbt[f, m]
                mels_ps = ps_m_pool.tile([128, RT * N_MELS], f32, name="mels_ps")
                for j in range(RT):
                    for c in range(NFC):
                        fs = f_sizes[c]
                        nc.tensor.matmul(
                            mels_ps[:, j * N_MELS : (j + 1) * N_MELS],
                            mag_sb[:fs, c, j * 128 : (j + 1) * 128],
                            fbt_sb[:fs, c, :],
                            start=(c == 0),
                            stop=(c == NFC - 1),
                        )
                # exponent bits out of PSUM (rest of finalize is batched)
                nc.vector.tensor_scalar(
                    mac["e_sb"][:, gg * RT * N_MELS : (gg + 1) * RT * N_MELS],
                    mels_ps.bitcast(i32),
                    23,
                    None,
                    mybir.AluOpType.logical_shift_right,
                )
                if gg == mac["GPM"] - 1:
                    # finalize: floor(log2(m)) = max(bits >> 23, 75) - 127
                    JT = mac["JT"]
                    e_sb = mac["e_sb"]
                    ef_sb = fin_pool.tile(
                        [128, JT * N_MELS], f32, tag="ef_sb", name="ef_sb"
                    )
                    nc.vector.tensor_copy(ef_sb, e_sb)
                    o_sb = fin_pool.tile(
                        [128, JT * N_MELS], f32, tag="o_sb", name="o_sb"
                    )
                    nc.vector.tensor_scalar(
                        o_sb,
                        ef_sb,
                        75.0,
                        127.0,
                        mybir.AluOpType.max,
                        mybir.AluOpType.subtract,
                    )
                    # store: one DMA per macro, JT rows per partition
                    nc.scalar.dma_start(
                        out=out_d[
                            mac["m0"] : mac["m0"] + JT * 128, :
                        ].rearrange("(p j) m -> p (j m)", j=JT),
                        in_=o_sb,
                    )

            for g in range(len(groups)):
                stage_T(g)
                stage_M1(g)
                stage_M2(g)
    nc.compile()
    return nc


def _prep_weights(filter_banks, hw):
    """Host-side: cosine*window matrix (bf16) and transposed filterbank."""
    fb = np.asarray(filter_banks, dtype=np.float32)
    n_mels, n_bins = fb.shape  # (20, 257)
    assert n_mels == N_MELS and n_bins == FRAME // 2 + 1

    if np.all(fb[:, 0] == 0.0):
        k0 = 1  # DC bin unused by the filterbank (structurally true)
    else:
        k0 = 0
    NF = n_bins - k0

    n = np.arange(FRAME, dtype=np.float64)
    k = np.arange(k0, n_bins, dtype=np.float64)
    C = np.cos(2.0 * np.pi * np.outer(n, k) / FRAME) / FRAME
    W = (np.asarray(hw, dtype=np.float64)[:, None] * C).astype(ml_dtypes.bfloat16)
    NQ = FRAME // 128
    w_chunks = np.ascontiguousarray(W.reshape(NQ, 128, NF))

    NFC = _ceil_div(NF, 128)
    fbt = np.zeros((NFC, 128, N_MELS), dtype=ml_dtypes.bfloat16)
    fbT = fb[:, k0:].T.astype(ml_dtypes.bfloat16)  # [NF, 20]
    for c in range(NFC):
        fs = min(128, NF - 128 * c)
        fbt[c, :fs, :] = fbT[128 * c : 128 * c + fs, :]
    return w_chunks, fbt, NF


_CACHE = {}


def _get_graph(R, NF, group_r, xbar_mod):
    key = (R, NF, group_r, xbar_mod)
    if key not in _CACHE:
        _CACHE[key] = build_graph(R, NF, group_r, xbar_mod)
    return _CACHE[key]


def kernel(inputs, filter_banks, hw, _trace=False, _group_r=512, _xbar_mod=2):
    x = np.ascontiguousarray(np.asarray(inputs, dtype=np.float32))
    assert x.shape == (B, T, FRAME), x.shape
    w_chunks, fbt, NF = _prep_weights(filter_banks, hw)

    shards = x.reshape(N_CORES, B // N_CORES * T, FRAME)
    nc = _get_graph(R_PER_CORE, NF, _group_r, _xbar_mod)
    in_maps = [
        {"x": shards[i], "w": w_chunks, "fbt": fbt} for i in range(N_CORES)
    ]
    res = run_bass_kernel_spmd(
        nc, in_maps, core_ids=list(range(N_CORES)), trace=_trace
    )
    out = np.stack([res.results[i]["out"] for i in range(N_CORES)], axis=0)
    out = out.reshape(B, T, N_MELS, 1).astype(np.float32)
    if _trace:
        kernel._last_result = res
    return out


# revision 12
# speedup vs baseline: 1.1807x; 1.0297x over previous
"""Trainium2 Bass kernel for AudioPreprocessingLayer.

Computes: floor(log2(mel_fb @ (rfft(x*hamming, norm=forward).real ** 2)))
for x of shape (4096, 32, 512), sharded batch-wise across 8 NeuronCores.

Key ideas:
  - rfft(.).real is a matmul with the cosine matrix C[n,k] = cos(2*pi*k*n/512)/512.
    The hamming window folds into it host-side: W = diag(hw) @ C, stored bf16.
  - Mel filterbank column 0 (DC bin) is structurally zero, so only bins 1..256
    are computed -> 256 = 2x128 clean chunks (checked at runtime, with a
    257-bin fallback).
  - x is cast once to bf16 by the load DMA. The row transpose (rows -> n on
    partitions) is split between two resources that run concurrently:
      * DMA XBAR (InstDmaTransposeAnt) on the sync queue: one instruction
        transposes a whole macro [128p, JT, 512n] -> [128 n', (j,q), 128 p].
      * PE identity matmuls (bf16) + PSUM->SBUF bf16 copies round-robined
        across the vector/scalar/gpsimd engines.
  - floor(log2(m)) for positive fp32 m is exactly
    max(bitcast_int32(m) >> 23, 75) - 127   (the max() also maps the
    mels==0 -> eps=2^-52 case to -52 exactly).
  - Rows are mapped to partitions in blocks of JT per macro-group
    (row = m0 + JT*p + j), so every load DMA descriptor covers JT consecutive
    DRAM rows (up to 32 KB), and both transpose paths produce the same
    r_local = j*128 + p ordering, matching one store DMA per macro.
"""

import os
import sys

for _p in ("/opt/trn_rl_repo",):
    if _p not in sys.path and os.path.isdir(_p):
        sys.path.append(_p)

import numpy as np
import ml_dtypes

import concourse.bass as bass
from concourse import bacc, mybir
from concourse.tile import TileContext
from concourse.bass_utils import run_bass_kernel_spmd
from concourse.masks import make_identity

N_CORES = 8
B, T, FRAME = 4096, 32, 512
R_PER_CORE = (B // N_CORES) * T  # 16384 rows of length 512 per core
N_MELS = 20

f32 = mybir.dt.float32
bf16 = mybir.dt.bfloat16
i32 = mybir.dt.int32


def _ceil_div(a, b):
    return (a + b - 1) // b


def build_graph(R=R_PER_CORE, NF=256, group_r=512, xbar_mod=2):
    """Build the SPMD Bass graph for one core's shard.

    x:   [R, 512]  f32   rows to transform
    w:   [4, 128, NF] bf16  cosine*window matrix, chunked along n
    fbt: [NFC, 128, N_MELS] bf16  mel filterbank transposed+chunked along freq
    out: [R, N_MELS] f32

    Macro mg uses the xbar transpose iff xbar_mod and mg % xbar_mod == 0,
    else the PE transpose path.
    """
    assert R % group_r == 0 and group_r % 128 == 0
    RT = group_r // 128          # row subtiles per group (4)
    NQ = FRAME // 128            # 4 n-chunks
    NFC = _ceil_div(NF, 128)     # freq chunks
    f_sizes = [min(128, NF - 128 * c) for c in range(NFC)]

    nc = bacc.Bacc(None, target_bir_lowering=False)
    x_d = nc.declare_dram_parameter("x", [R, FRAME], f32, isOutput=False)
    w_d = nc.declare_dram_parameter("w", [NQ, 128, NF], bf16, isOutput=False)
    fbt_d = nc.declare_dram_parameter("fbt", [NFC, 128, N_MELS], bf16, isOutput=False)
    out_d = nc.declare_dram_parameter("out", [R, N_MELS], f32, isOutput=True)

    with TileContext(nc) as tc:
        with (
            tc.tile_pool(name="consts", bufs=1) as consts,
            tc.tile_pool(name="xb", bufs=3) as xb_pool,
            tc.tile_pool(name="xt", bufs=2) as xt_pool,
            tc.tile_pool(name="xq", bufs=2) as xq_pool,
            tc.tile_pool(name="mag", bufs=3) as mag_pool,
            tc.tile_pool(name="fin", bufs=3) as fin_pool,
            tc.tile_pool(name="ps_xt", bufs=3, space="PSUM") as ps_xt_pool,
            tc.tile_pool(name="ps_y", bufs=2, space="PSUM") as ps_y_pool,
            tc.tile_pool(name="ps_m", bufs=1, space="PSUM") as ps_m_pool,
        ):
            # ---- constants (bf16 straight from DRAM, no casts) ----
            ident = consts.tile([128, 128], bf16)
            make_identity(nc, ident)
            w_sb = consts.tile([128, NQ, NF], bf16)
            nc.sync.dma_start(out=w_sb, in_=w_d.rearrange("q p f -> p q f"))
            fbt_sb = consts.tile([128, NFC, N_MELS], bf16)
            nc.sync.dma_start(out=fbt_sb, in_=fbt_d.rearrange("c p m -> p c m"))

            # macro-group sizes (in groups of group_r rows): small at the head
            # so the pipeline fills quickly, small at the tail to shrink the
            # drain.
            n_groups = R // group_r
            gpm = 2048 // group_r
            head = [1, 1, 2]
            tail = [max(1, gpm // 2)] * 4
            mid = n_groups - sum(head) - sum(tail)
            if mid >= gpm:
                body = [gpm] * (mid // gpm)
                if mid % gpm:
                    body.append(mid % gpm)
                gpm_list = head + body + tail
            else:
                gpm_list = [1] * n_groups
            assert sum(gpm_list) == n_groups, (gpm_list, n_groups)

            groups = []   # (macro, gg) per group
            macros = []   # per macro: dict(m0, GPM, JT)
            m0 = 0
            for mg, GPM in enumerate(gpm_list):
                use_xbar = bool(xbar_mod) and (mg % xbar_mod == 0)
                macros.append(
                    {"m0": m0, "GPM": GPM, "JT": GPM * RT, "xbar": use_xbar}
                )
                for gg in range(GPM):
                    groups.append((mg, gg))
                m0 += GPM * group_r

            st = {}
            cp_engines = None  # set lazily (round-robin for PSUM->SBUF copies)

            def stage_T(g):
                mg, gg = groups[g]
                mac = macros[mg]
                if gg == 0:
                    JT = mac["JT"]
                    # casting DMA (f32 dram -> bf16 sbuf); row m0 + JT*p + j
                    # -> partition p, slot j (big contiguous descriptors)
                    xb_sb = xb_pool.tile([128, JT, FRAME], bf16, name="xb_sb")
                    nc.gpsimd.dma_start(
                        out=xb_sb,
                        in_=x_d[
                            mac["m0"] : mac["m0"] + JT * 128, :
                        ].rearrange("(p j) n -> p j n", j=JT),
                    )
                    mac["xb"] = xb_sb
                    if mac["xbar"]:
                        # ONE xbar transpose for the whole macro:
                        # xt[n', (j, q), p] with n = q*128 + n'
                        xt_sb = xt_pool.tile(
                            [128, JT, NQ, 128], bf16, name="xt_sb"
                        )
                        nc.sync.dma_start(out=xt_sb, in_=xb_sb, transpose=True)
                        mac["xt"] = xt_sb
                    mac["e_sb"] = fin_pool.tile(
                        [128, JT * N_MELS], i32, tag="e_sb", name="e_sb"
                    )
                if not mac["xbar"]:
                    # PE transpose of this group's RT row-blocks; PSUM->SBUF
                    # bf16 copies round-robin across vector/scalar/gpsimd
                    nonlocal cp_engines
                    if cp_engines is None:
                        # gpsimd cannot read PSUM; vector gets 2/3 since
                        # scalar also runs the Square activations
                        cp_engines = [nc.vector, nc.scalar, nc.vector]
                    xb_sb = mac["xb"]
                    xq_sb = []
                    for q in range(NQ):
                        t = ps_xt_pool.tile(
                            [128, group_r], f32, name=f"xt{q}", tag="xt"
                        )
                        for j in range(RT):
                            nc.tensor.matmul(
                                t[:, j * 128 : (j + 1) * 128],
                                xb_sb[:, gg * RT + j, q * 128 : (q + 1) * 128],
                                ident,
                                start=True,
                                stop=True,
                            )
                        dst = xq_pool.tile(
                            [128, group_r], bf16, name=f"xq{q}", tag=f"xq{q}"
                        )
                        xq_sb.append(dst)
                        eng = cp_engines[(g * NQ + q) % 3]
                        if eng is nc.scalar:
                            eng.copy(dst, t)
                        else:
                            eng.tensor_copy(dst, t)
                    st[("xq", g)] = xq_sb

            def stage_M1(g):
                # matmul 1: yT[f, r] += W[n, f].T @ xT[n, r]
                mg, gg = groups[g]
                mac = macros[mg]
                y_ps = ps_y_pool.tile([128, NFC, group_r], f32, name="y_ps")
                if mac["xbar"]:
                    xt_sb = mac["xt"]
                    movings = [
                        xt_sb[:, gg * RT : (gg + 1) * RT, q, :] for q in range(NQ)
                    ]
                else:
                    movings = st.pop(("xq", g))
                # c innermost: consecutive matmuls alternate PSUM banks,
                # enabling cross-bank ILP in the PE drain path
                for q in range(NQ):
                    for c in range(NFC):
                        fs = f_sizes[c]
                        nc.tensor.matmul(
                            y_ps[:fs, c, :],
                            w_sb[:, q, 128 * c : 128 * c + fs],
                            movings[q],
                            start=(q == 0),
                            stop=(q == NQ - 1),
                        )
                # square: magT = yT*yT (fused, psum -> sbuf bf16)
                mag_sb = mag_pool.tile([128, NFC, group_r], bf16, name="mag_sb")
                nc.scalar.activation(
                    mag_sb, y_ps, mybir.ActivationFunctionType.Square
                )
                st[("mag", g)] = mag_sb

            def stage_M2(g):
                mg, gg = groups[g]
                mac = macros[mg]
                mag_sb = st.pop(("mag", g))
                # matmul 2: mels[r, m] += magT[f, r].T @ fbt[f, m]
                mels_ps = ps_m_pool.tile([128, RT * N_MELS], f32, name="mels_ps")
                # j innermost: the fbt stationary is loaded once per c and
                # reused across the RT row-blocks (has_written accumulation is
                # per element, so chain flags depend only on c)
                # start=True clears has_written for the WHOLE bank, so it may
                # only appear on the first matmul touching this bank; j>0
                # c==0 matmuls overwrite via their cleared has_written bits
                for c in range(NFC):
                    fs = f_sizes[c]
                    for j in range(RT):
                        nc.tensor.matmul(
                            mels_ps[:, j * N_MELS : (j + 1) * N_MELS],
                            mag_sb[:fs, c, j * 128 : (j + 1) * 128],
                            fbt_sb[:fs, c, :],
                            start=(c == 0 and j == 0),
                            stop=(c == NFC - 1 and j == RT - 1),
                        )
                # exponent bits out of PSUM (rest of finalize is batched)
                nc.vector.tensor_scalar(
                    mac["e_sb"][:, gg * RT * N_MELS : (gg + 1) * RT * N_MELS],
                    mels_ps.bitcast(i32),
                    23,
                    None,
                    mybir.AluOpType.logical_shift_right,
                )
                if gg == mac["GPM"] - 1:
                    # finalize: floor(log2(m)) = max(bits >> 23, 75) - 127
                    JT = mac["JT"]
                    e_sb = mac["e_sb"]
                    ef_sb = fin_pool.tile(
                        [128, JT * N_MELS], f32, tag="ef_sb", name="ef_sb"
                    )
                    nc.vector.tensor_copy(ef_sb, e_sb)
                    o_sb = fin_pool.tile(
                        [128, JT * N_MELS], f32, tag="o_sb", name="o_sb"
                    )
                    nc.vector.tensor_scalar(
                        o_sb,
                        ef_sb,
                        75.0,
                        127.0,
                        mybir.AluOpType.max,
                        mybir.AluOpType.subtract,
                    )
                    # store: one DMA per macro, JT rows per partition
                    nc.scalar.dma_start(
                        out=out_d[
                            mac["m0"] : mac["m0"] + JT * 128, :
                        ].rearrange("(p j) m -> p (j m)", j=JT),
                        in_=o_sb,
                    )

            for g in range(len(groups)):
                stage_T(g)
                stage_M1(g)
                stage_M2(g)
    nc.compile()
    return nc


def _prep_weights(filter_banks, hw):
    """Host-side: cosine*window matrix (bf16) and transposed filterbank."""
    fb = np.asarray(filter_banks, dtype=np.float32)
    n_mels, n_bins = fb.shape  # (20, 257)
    assert n_mels == N_MELS and n_bins == FRAME // 2 + 1

    if np.all(fb[:, 0] == 0.0):
        k0 = 1  # DC bin unused by the filterbank (structurally true)
    else:
        k0 = 0
    NF = n_bins - k0

    n = np.arange(FRAME, dtype=np.float64)
    k = np.arange(k0, n_bins, dtype=np.float64)
    C = np.cos(2.0 * np.pi * np.outer(n, k) / FRAME) / FRAME
    W = (np.asarray(hw, dtype=np.float64)[:, None] * C).astype(ml_dtypes.bfloat16)
    NQ = FRAME // 128
    w_chunks = np.ascontiguousarray(W.reshape(NQ, 128, NF))

    NFC = _ceil_div(NF, 128)
    fbt = np.zeros((NFC, 128, N_MELS), dtype=ml_dtypes.bfloat16)
    fbT = fb[:, k0:].T.astype(ml_dtypes.bfloat16)  # [NF, 20]
    for c in range(NFC):
        fs = min(128, NF - 128 * c)
        fbt[c, :fs, :] = fbT[128 * c : 128 * c + fs, :]
    return w_chunks, fbt, NF


_CACHE = {}


def _get_graph(R, NF, group_r, xbar_mod):
    key = (R, NF, group_r, xbar_mod)
    if key not in _CACHE:
        _CACHE[key] = build_graph(R, NF, group_r, xbar_mod)
    return _CACHE[key]


def kernel(inputs, filter_banks, hw, _trace=False, _group_r=512, _xbar_mod=2):
    x = np.ascontiguousarray(np.asarray(inputs, dtype=np.float32))
    assert x.shape == (B, T, FRAME), x.shape
    w_chunks, fbt, NF = _prep_weights(filter_banks, hw)

    shards = x.reshape(N_CORES, B // N_CORES * T, FRAME)
    nc = _get_graph(R_PER_CORE, NF, _group_r, _xbar_mod)
    in_maps = [
        {"x": shards[i], "w": w_chunks, "fbt": fbt} for i in range(N_CORES)
    ]
    res = run_bass_kernel_spmd(
        nc, in_maps, core_ids=list(range(N_CORES)), trace=_trace
    )
    out = np.stack([res.results[i]["out"] for i in range(N_CORES)], axis=0)
    out = out.reshape(B, T, N_MELS, 1).astype(np.float32)
    if _trace:
        kernel._last_result = res
    return out


# revision 13
# speedup vs baseline: 2.0286x; 1.7181x over previous
"""Trainium2 Bass kernel for AudioPreprocessingLayer.

Computes: floor(log2(mel_fb @ (rfft(x*hamming, norm=forward).real ** 2)))
for x of shape (4096, 32, 512), sharded batch-wise across 8 NeuronCores.

Key ideas:
  - rfft(.).real is a matmul with the cosine matrix C[n,k] = cos(2*pi*k*n/512)/512.
    The hamming window folds into it host-side: W = diag(hw) @ C, stored bf16.
  - Mel filterbank column 0 (DC bin) is structurally zero, so only bins 1..256
    are computed -> 256 = 2x128 clean chunks (checked at runtime, with a
    257-bin fallback).
  - x is cast once to bf16 by the load DMA; the on-chip transpose runs as
    REGULAR bf16 matmuls against an identity (1 cycle/row AND counts as PE
    activity, keeping the HAM clock gate at 2.4 GHz), then PSUM -> SBUF bf16
    copies split across the vector and scalar engines.
  - All loads stream on the gpsimd SWDGE queue with 7 macros of SBUF
    lookahead, so HBM never idles; all output stores are issued at the very
    end (from a persistent accumulator tile) so no store-semaphore wait ever
    blocks a compute queue.
  - floor(log2(m)) for positive fp32 m is exactly
    max(bitcast_int32(m) >> 23, 75) - 127   (the max() also maps the
    mels==0 -> eps=2^-52 case to -52 exactly).
  - Rows are mapped to partitions in blocks of JT per macro-group
    (row = m0 + JT*p + j), so every load DMA descriptor covers JT consecutive
    DRAM rows (16 KB in), matching one store DMA per macro.
"""

import os
import sys

for _p in ("/opt/trn_rl_repo",):
    if _p not in sys.path and os.path.isdir(_p):
        sys.path.append(_p)

import numpy as np
import ml_dtypes

import concourse.bass as bass
from concourse import bacc, mybir
from concourse.tile import TileContext
from concourse.bass_utils import run_bass_kernel_spmd
from concourse.masks import make_identity

N_CORES = 8
B, T, FRAME = 4096, 32, 512
R_PER_CORE = (B // N_CORES) * T  # 16384 rows of length 512 per core
N_MELS = 20

f32 = mybir.dt.float32
bf16 = mybir.dt.bfloat16
i32 = mybir.dt.int32


def _ceil_div(a, b):
    return (a + b - 1) // b


def build_graph(R=R_PER_CORE, NF=256, group_r=512):
    """Build the SPMD Bass graph for one core's shard.

    x:   [R, 512]  f32   rows to transform
    w:   [4, 128, NF] bf16  cosine*window matrix, chunked along n
    fbt: [NFC, 128, N_MELS] bf16  mel filterbank transposed+chunked along freq
    out: [R, N_MELS] f32
    """
    assert R % group_r == 0 and group_r % 128 == 0
    RT = group_r // 128          # row subtiles per group (4)
    NQ = FRAME // 128            # 4 n-chunks
    NFC = _ceil_div(NF, 128)     # freq chunks
    f_sizes = [min(128, NF - 128 * c) for c in range(NFC)]

    nc = bacc.Bacc(None, target_bir_lowering=False)
    x_d = nc.declare_dram_parameter("x", [R, FRAME], f32, isOutput=False)
    w_d = nc.declare_dram_parameter("w", [NQ, 128, NF], bf16, isOutput=False)
    fbt_d = nc.declare_dram_parameter("fbt", [NFC, 128, N_MELS], bf16, isOutput=False)
    out_d = nc.declare_dram_parameter("out", [R, N_MELS], f32, isOutput=True)

    with TileContext(nc) as tc:
        with (
            tc.tile_pool(name="consts", bufs=1) as consts,
            tc.tile_pool(name="xb", bufs=8) as xb_pool,
            tc.tile_pool(name="xq", bufs=2) as xq_pool,
            tc.tile_pool(name="mag", bufs=3) as mag_pool,
            tc.tile_pool(name="fin", bufs=3) as fin_pool,
            tc.tile_pool(name="ps_xt", bufs=3, space="PSUM") as ps_xt_pool,
            tc.tile_pool(name="ps_y", bufs=2, space="PSUM") as ps_y_pool,
            tc.tile_pool(name="ps_m", bufs=1, space="PSUM") as ps_m_pool,
        ):
            # ---- constants (bf16 straight from DRAM, no casts) ----
            ident = consts.tile([128, 128], bf16)
            make_identity(nc, ident)
            w_sb = consts.tile([128, NQ, NF], bf16)
            nc.sync.dma_start(out=w_sb, in_=w_d.rearrange("q p f -> p q f"))
            fbt_sb = consts.tile([128, NFC, N_MELS], bf16)
            nc.sync.dma_start(out=fbt_sb, in_=fbt_d.rearrange("c p m -> p c m"))
            # full per-core output staged in SBUF; stored at the very end
            o_all = consts.tile([128, (R // 128) * N_MELS], f32)

            # macro sizes (in groups): tiny head for fast ramp, tiny tail
            n_groups = R // group_r
            if n_groups >= 8:
                gpm_list = [1, 1] + [2] * ((n_groups - 4) // 2) + [1, 1]
            else:
                gpm_list = [1] * n_groups
            assert sum(gpm_list) == n_groups, (gpm_list, n_groups)

            groups = []   # (macro, gg) per group
            macros = []   # per macro: dict(m0, GPM, JT, off)
            m0 = 0
            for mg, GPM in enumerate(gpm_list):
                macros.append(
                    {"m0": m0, "GPM": GPM, "JT": GPM * RT,
                     "off": (m0 // 128) * N_MELS}
                )
                for gg in range(GPM):
                    groups.append((mg, gg))
                m0 += GPM * group_r

            st = {}
            cp_engines = [nc.vector, nc.scalar, nc.vector]

            def stage_T(g):
                mg, gg = groups[g]
                mac = macros[mg]
                if gg == 0:
                    JT = mac["JT"]
                    # casting DMA (f32 dram -> bf16 sbuf); row m0 + JT*p + j
                    # -> partition p, slot j (big contiguous descriptors)
                    xb_sb = xb_pool.tile([128, JT, FRAME], bf16, name="xb_sb")
                    nc.gpsimd.dma_start(
                        out=xb_sb,
                        in_=x_d[
                            mac["m0"] : mac["m0"] + JT * 128, :
                        ].rearrange("(p j) n -> p j n", j=JT),
                    )
                    mac["xb"] = xb_sb
                # PE transpose of this group's RT row-blocks; PSUM -> SBUF
                # bf16 copies (exact: x is bf16-valued) round-robin 2:1
                # vector:scalar
                xb_sb = mac["xb"]
                xq_sb = []
                for q in range(NQ):
                    t = ps_xt_pool.tile(
                        [128, group_r], f32, name=f"xt{q}", tag="xt"
                    )
                    for j in range(RT):
                        nc.tensor.matmul(
                            t[:, j * 128 : (j + 1) * 128],
                            xb_sb[:, gg * RT + j, q * 128 : (q + 1) * 128],
                            ident,
                            start=True,
                            stop=True,
                        )
                    dst = xq_pool.tile(
                        [128, group_r], bf16, name=f"xq{q}", tag=f"xq{q}"
                    )
                    xq_sb.append(dst)
                    eng = cp_engines[(g * NQ + q) % 3]
                    if eng is nc.scalar:
                        eng.copy(dst, t)
                    else:
                        eng.tensor_copy(dst, t)
                st[("xq", g)] = xq_sb

            def stage_M1(g):
                # matmul 1: yT[f, r] += W[n, f].T @ xT[n, r]
                y_ps = ps_y_pool.tile([128, NFC, group_r], f32, name="y_ps")
                movings = st.pop(("xq", g))
                # c innermost: consecutive matmuls alternate PSUM banks
                for q in range(NQ):
                    for c in range(NFC):
                        fs = f_sizes[c]
                        nc.tensor.matmul(
                            y_ps[:fs, c, :],
                            w_sb[:, q, 128 * c : 128 * c + fs],
                            movings[q],
                            start=(q == 0),
                            stop=(q == NQ - 1),
                        )
                # square: magT = yT*yT (fused, psum -> sbuf bf16)
                mag_sb = mag_pool.tile([128, NFC, group_r], bf16, name="mag_sb")
                nc.scalar.activation(
                    mag_sb, y_ps, mybir.ActivationFunctionType.Square
                )
                st[("mag", g)] = mag_sb

            def stage_M2(g):
                mg, gg = groups[g]
                mac = macros[mg]
                mag_sb = st.pop(("mag", g))
                # matmul 2: mels[r, m] += magT[f, r].T @ fbt[f, m]
                mels_ps = ps_m_pool.tile([128, RT * N_MELS], f32, name="mels_ps")
                # j innermost reuses the fbt stationary; start=True only on
                # the first matmul into the bank (start clears has_written
                # for the WHOLE bank)
                for c in range(NFC):
                    fs = f_sizes[c]
                    for j in range(RT):
                        nc.tensor.matmul(
                            mels_ps[:, j * N_MELS : (j + 1) * N_MELS],
                            mag_sb[:fs, c, j * 128 : (j + 1) * 128],
                            fbt_sb[:fs, c, :],
                            start=(c == 0 and j == 0),
                            stop=(c == NFC - 1 and j == RT - 1),
                        )
                # finalize floor(log2(m)) = max(bits >> 23, 75) - 127 straight
                # into the persistent output tile (all on vector)
                e_sb = fin_pool.tile([128, RT * N_MELS], i32, tag="e_sb", name="e_sb")
                nc.vector.tensor_scalar(
                    e_sb,
                    mels_ps.bitcast(i32),
                    23,
                    None,
                    mybir.AluOpType.logical_shift_right,
                )
                ef_sb = fin_pool.tile([128, RT * N_MELS], f32, tag="ef_sb", name="ef_sb")
                nc.vector.tensor_copy(ef_sb, e_sb)
                o_off = mac["off"] + gg * RT * N_MELS
                nc.vector.tensor_scalar(
                    o_all[:, o_off : o_off + RT * N_MELS],
                    ef_sb,
                    75.0,
                    127.0,
                    mybir.AluOpType.max,
                    mybir.AluOpType.subtract,
                )

            for g in range(len(groups)):
                stage_T(g)
                stage_M1(g)
                stage_M2(g)

            # ---- all stores at the end: one DMA per macro ----
            for mac in macros:
                JT = mac["JT"]
                nc.sync.dma_start(
                    out=out_d[
                        mac["m0"] : mac["m0"] + JT * 128, :
                    ].rearrange("(p j) m -> p (j m)", j=JT),
                    in_=o_all[:, mac["off"] : mac["off"] + JT * N_MELS],
                )
    nc.compile()
    return nc


def _prep_weights(filter_banks, hw):
    """Host-side: cosine*window matrix (bf16) and transposed filterbank."""
    fb = np.asarray(filter_banks, dtype=np.float32)
    n_mels, n_bins = fb.shape  # (20, 257)
    assert n_mels == N_MELS and n_bins == FRAME // 2 + 1

    if np.all(fb[:, 0] == 0.0):
        k0 = 1  # DC bin unused by the filterbank (structurally true)
    else:
        k0 = 0
    NF = n_bins - k0

    n = np.arange(FRAME, dtype=np.float64)
    k = np.arange(k0, n_bins, dtype=np.float64)
    C = np.cos(2.0 * np.pi * np.outer(n, k) / FRAME) / FRAME
    W = (np.asarray(hw, dtype=np.float64)[:, None] * C).astype(ml_dtypes.bfloat16)
    NQ = FRAME // 128
    w_chunks = np.ascontiguousarray(W.reshape(NQ, 128, NF))

    NFC = _ceil_div(NF, 128)
    fbt = np.zeros((NFC, 128, N_MELS), dtype=ml_dtypes.bfloat16)
    fbT = fb[:, k0:].T.astype(ml_dtypes.bfloat16)  # [NF, 20]
    for c in range(NFC):
        fs = min(128, NF - 128 * c)
        fbt[c, :fs, :] = fbT[128 * c : 128 * c + fs, :]
    return w_chunks, fbt, NF


_CACHE = {}


def _get_graph(R, NF, group_r):
    key = (R, NF, group_r)
    if key not in _CACHE:
        _CACHE[key] = build_graph(R, NF, group_r)
    return _CACHE[key]


def kernel(inputs, filter_banks, hw, _trace=False, _group_r=512):
    x = np.ascontiguousarray(np.asarray(inputs, dtype=np.float32))
    assert x.shape == (B, T, FRAME), x.shape
    w_chunks, fbt, NF = _prep_weights(filter_banks, hw)

    shards = x.reshape(N_CORES, B // N_CORES * T, FRAME)
    nc = _get_graph(R_PER_CORE, NF, _group_r)
    in_maps = [
        {"x": shards[i], "w": w_chunks, "fbt": fbt} for i in range(N_CORES)
    ]
    res = run_bass_kernel_spmd(
        nc, in_maps, core_ids=list(range(N_CORES)), trace=_trace
    )
    out = np.stack([res.results[i]["out"] for i in range(N_CORES)], axis=0)
    out = out.reshape(B, T, N_MELS, 1).astype(np.float32)
    if _trace:
        kernel._last_result = res
    return out
